# revision 46
# baseline (speedup 1.0000x reference)
"""Trainium2 Bass kernel for AdaptiveModalitySelectionSystem (moe_routing).

Data-parallel over batch B=4096 across 8 NeuronCores (B_local=512 each).

Host-side preprocessing moves all layout/dtype work off the device:
  - x cast to bf16, prepacked to [K, 128, DCH, BL] (partition-major) so each
    DMA descriptor is a 2-4KB contiguous run.
  - W_enc cast to bf16, prepacked to [K, 128, DCH, H].
  - context transposed + packed together with W1 into one tensor CW
    [128, CCH, 576] (cols 0:512 ctx^T chunk, 512:576 W1 chunk) -> one DMA.
  - W2/W3 packed into Q [64, 36]; b1/g_ln/beta_ln/b2/b3p + gumbel + the
    host-folded softmax(fusion_w) packed into one PG [128, 28] tensor.

Device schedule (v2 — single need-ordered input stream):
  - ALL x/W_enc input DMAs ride ONE queue (sync), interleaved in exact
    consumption order (x(k,c-pair) then w(k,c-pair)).  With two queues the
    16 DMA engines round-robin between them, which halved the W stream's
    bandwidth and starved the PE mid-k (measured 2.1us + 1.5us gaps).
  - CW + b_enc on the scalar queue, PG/Q on the vector queue: the router's
    inputs land by ~+2.5us independent of the big stream.
  - PE order: warm-up transposes -> router GEMM1 -> k=0 encoder in c-major
    order over bt-pairs (matches DMA arrival; only 4 PSUM banks live) with
    the rest of the router (LN stats, GEMM2/3, logit/coef transposes)
    slotted between c-groups -> k=1..3 bt-major with bias matmuls folded in.
  - k=0 PSUM drains are plain copies on the otherwise-idle GpSimd engine
    (coef not ready yet); coef0 is folded later via
    acc = acc*coef0 + coefT^T @ b_enc.  k>=1 drains are DVE
    scalar_tensor_tensor acc = psum*coef_k + acc.
  - LayerNorm stats as PE matmuls against an all-1/64 stationary matrix so
    mean/meansq are replicated over partitions (no single-lane DVE chains).
No collectives: each core computes its own output shard independently.
"""
from contextlib import ExitStack

import numpy as np
import ml_dtypes

import concourse.bass as bass
import concourse.tile as tile
from concourse import bacc, mybir
from concourse.bass_utils import run_bass_kernel_spmd

N_CORES = 8
B, K, D, H, CTX, RH = 4096, 4, 1024, 1024, 256, 64
RH2 = RH // 2
BL = B // N_CORES  # 512 rows per core
NBT = BL // 128    # 4 batch tiles per core
DCH = D // 128     # 8 contraction chunks per modality
CCH = CTX // 128   # 2 contraction chunks for the router
HB = 512           # h-block width (one PSUM bank)
NHB = H // HB      # 2 h-blocks
N_WARM = 7         # PE warm-up transposes
EPS = 1e-5
F32 = mybir.dt.float32
BF16 = mybir.dt.bfloat16
F32R = mybir.dt.float32r
AF = mybir.ActivationFunctionType
OP = mybir.AluOpType
AX = mybir.AxisListType
_BF = ml_dtypes.bfloat16

# PG column layout: [b1|g_ln|b_ln|b2|b3p] (5), gumbel (NBT*K=16), w4 (K)
PGC = 5 + NBT * K + K


def _build():
    nc = bacc.Bacc("TRN2", target_bir_lowering=False, debug=False,
                   num_devices=N_CORES)

    def din(name, shape, dt=F32):
        return nc.dram_tensor(name, shape, dt, kind="ExternalInput").ap()

    CW_e = din("CW", [128, CCH, 512 + RH], F32R)
    xP_e = din("xP", [K, 128, DCH, BL], BF16)
    wP_e = din("WP", [K, 128, DCH, H], BF16)
    PG_e = din("PG", [128, PGC])
    Q_e = din("Q", [RH, RH2 + K], F32R)
    be_e = din("b_encP", [K, H], BF16)
    id_e = din("ident", [128, 128])
    out_e = nc.dram_tensor("out", [BL, H], F32, kind="ExternalOutput").ap()

    with tile.TileContext(nc) as tc, ExitStack() as st:
        singles = st.enter_context(tc.tile_pool(name="singles", bufs=1))
        rt = st.enter_context(tc.tile_pool(name="rt", bufs=2))
        psg = st.enter_context(tc.tile_pool(name="psg", bufs=6, space="PSUM"))
        pst = st.enter_context(tc.tile_pool(name="pst", bufs=2, space="PSUM"))

        # ---- constants ----
        # junk: memset-initialized source for the PE warm-up transposes.
        # A DMA-fed source measured +3..+7us before the first warm-up could
        # issue (per-transfer DMA latency); a memset is ready at ~+0.3us.
        junk = singles.tile([128, 128], F32)
        nc.vector.memset(junk[:], 0.0)
        eps64 = singles.tile([RH, 1], F32)
        nc.vector.memset(eps64[:], EPS)
        inv64_f = singles.tile([RH, RH], F32)
        nc.vector.memset(inv64_f[:], 1.0 / RH)
        inv64 = singles.tile([RH, RH], F32R)
        nc.vector.tensor_copy(out=inv64[:], in_=inv64_f[:])

        # ---- input DMAs ----
        # scalar: router GEMM input + small packs.  They ride hardware
        # queues distinct from the sync stream's, so they transfer
        # concurrently with the first x/w chunks (the 8 HWDGE queues are
        # assigned round-robin across issuing engines).  The real identity
        # (for the logit/coef transposes, first use ~+15us) goes last.
        CW = singles.tile([128, CCH, 512 + RH], F32R)
        nc.scalar.dma_start(out=CW[:], in_=CW_e[:])
        PG = singles.tile([128, PGC], F32)
        nc.scalar.dma_start(out=PG[:], in_=PG_e[:])
        Q = singles.tile([RH, RH2 + K], F32R)
        nc.scalar.dma_start(out=Q[:], in_=Q_e[:])
        benc_sb = singles.tile([K, H], BF16)
        nc.scalar.dma_start(out=benc_sb[:], in_=be_e[:])
        ident = singles.tile([128, 128], F32)
        nc.scalar.dma_start(out=ident[:], in_=id_e[:])

        b1_c = PG[0:RH, 0:1]
        gln_c = PG[0:RH, 1:2]
        bln_c = PG[0:RH, 2:3]
        b2_c = PG[0:RH2, 3:4]
        b3p_c = PG[0:K, 4:5]
        gum_sb = PG[:, 5:5 + NBT * K].rearrange("p (t k) -> p t k", t=NBT)
        w4 = PG[:, 5 + NBT * K:PGC]

        acc = singles.tile([128, NBT, H], F32)
        coef = singles.tile([128, NBT, K], F32)
        coefT = singles.tile([K, NBT, 128], BF16)

        xs, ws = [], []
        for k in range(K):
            xst = singles.tile([128, DCH, BL], BF16, name=f"xs{k}")
            wst = singles.tile([128, DCH, H], BF16, name=f"ws{k}")
            xs.append(xst)
            ws.append(wst)

        # Single need-ordered stream on sync: x chunk(s), then the matching
        # w chunk(s), per k.  (Two queues split the 16 DMA engines and
        # halve each stream's rate; need-order on one queue self-paces.)
        # k=0's head is fine-grained so the first encoder matmuls unblock
        # ASAP; the bulk rides in 0.5-2MB transfers (8KB descriptor runs)
        # to keep the DMA rings deeply queued through the ramp.
        for cs in (slice(0, 1), slice(1, 2), slice(2, 4), slice(4, 6),
                   slice(6, 8)):
            nc.sync.dma_start(out=xs[0][:, cs, :], in_=xP_e[0, :, cs, :])
            nc.sync.dma_start(out=ws[0][:, cs, :], in_=wP_e[0, :, cs, :])
        for k in range(1, K):
            for q in range(4):
                cs = slice(2 * q, 2 * q + 2)
                nc.sync.dma_start(out=xs[k][:, cs, :], in_=xP_e[k, :, cs, :])
                nc.sync.dma_start(out=ws[k][:, cs, :], in_=wP_e[k, :, cs, :])

        # ---- PE warm-up: cover until CW / first x,w chunks land ----
        # One shared PSUM target, written repeatedly: WAW on the in-order PE
        # costs nothing and avoids burning pst pool slots per warm-up.
        wps = pst.tile([128, 128], F32, tag="ps", name="warm")

        def warm():
            nc.tensor.transpose(out=wps[:], in_=junk[:], identity=junk[:])

        for i in range(N_WARM):
            warm()

        # ---- encoder GEMM blocks ----
        # k=0 runs c-major over bt-pairs (4 live PSUM banks) so consumption
        # matches the DMA arrival order; the remaining router PE ops are
        # slotted between c-groups.  k>=1 runs bt-major (data resident).
        # (pair tiles are allocated lazily: pair (2,3) must be claimed only
        # after pair (0,1)'s drains are emitted, or the pool's WAR recycling
        # would gate (2,3)'s matmuls on later-emitted readers.)
        k0_pms = {}

        def k0_alloc(pair):
            for bt in pair:
                k0_pms[bt] = [
                    psg.tile([128, HB], F32, tag="pm", name=f"pm0_{bt}_{hb}")
                    for hb in range(NHB)]

        def k0_cgroup(pair, q):
            for bt in pair:
                for c in (2 * q, 2 * q + 1):
                    for hb in range(NHB):
                        nc.tensor.matmul(
                            out=k0_pms[bt][hb][:],
                            lhsT=xs[0][:, c, bt * 128:(bt + 1) * 128],
                            rhs=ws[0][:, c, hb * HB:(hb + 1) * HB],
                            start=(c == 0), stop=(c == DCH - 1))

        def drain_k0(bt):
            # plain PSUM->SBUF copies on the (lightly-loaded) Scalar engine
            # (GpSimd cannot access PSUM); coef0 is folded in later via the
            # bias pass
            for hb in range(NHB):
                hsl = slice(hb * HB, (hb + 1) * HB)
                nc.scalar.activation(out=acc[:, bt, hsl],
                                     in_=k0_pms[bt][hb][:], func=AF.Copy)

        # --- k0 bt-pair (0,1), with the router overlapped.  The router's
        # first GEMM comes after the first c-groups so a late CW cannot
        # stall the (earlier-arriving) encoder stream at the in-order PE
        # queue.  Warm-up fills bridge the measured ~2.4us wait for w0c1 so
        # the HAM activity window stays busy (a fill runs while the DMA is
        # in flight; idle would push the 2.4GHz unthrottle point out past
        # the whole k=0 pass).
        k0_alloc((0, 1))

        def k0_c(pair, c):
            for bt in pair:
                for hb in range(NHB):
                    nc.tensor.matmul(
                        out=k0_pms[bt][hb][:],
                        lhsT=xs[0][:, c, bt * 128:(bt + 1) * 128],
                        rhs=ws[0][:, c, hb * HB:(hb + 1) * HB],
                        start=(c == 0), stop=(c == DCH - 1))

        k0_c((0, 1), 0)
        # cover the c1-chunk DMA wait: a ~2.2-2.3us PE gap here survives on
        # every core in every measured config — fill it almost entirely
        # (fills are ~220ns transposes; data has never arrived early enough
        # for them to cost anything)
        for _ in range(14):
            warm()
        k0_c((0, 1), 1)
        for _ in range(3):
            warm()
        k0_c((0, 1), 2)
        k0_c((0, 1), 3)

        # ---- router part 1: hT = (ctx @ W1 + b1)^T ----
        hps = pst.tile([RH, BL], F32, tag="ps")
        nc.tensor.matmul(out=hps[:], lhsT=CW[:, 0, 512:512 + RH],
                         rhs=CW[:, 0, 0:512], start=True, stop=False)
        nc.tensor.matmul(out=hps[:], lhsT=CW[:, 1, 512:512 + RH],
                         rhs=CW[:, 1, 0:512], start=False, stop=True)

        k0_c((0, 1), 4)

        # LN stat matmuls (mean / mean-square REPLICATED over partitions)
        hT_raw = rt.tile([RH, BL], F32R, tag="hT_raw")
        nc.vector.tensor_scalar_add(out=hT_raw[:], in0=hps[:], scalar1=b1_c)
        hsq = rt.tile([RH, BL], F32R, tag="hsq")
        nc.vector.tensor_tensor(out=hsq[:], in0=hT_raw[:], in1=hT_raw[:],
                                op=OP.mult)
        mups = pst.tile([RH, BL], F32, tag="ps", name="mups")
        nc.tensor.matmul(out=mups[:], lhsT=inv64[:], rhs=hT_raw[:],
                         start=True, stop=True)
        msps = pst.tile([RH, BL], F32, tag="ps", name="msps")
        nc.tensor.matmul(out=msps[:], lhsT=inv64[:], rhs=hsq[:],
                         start=True, stop=True)

        # router part 2 (DVE/ACT): var, rstd, normalize
        mu_sb = rt.tile([RH, BL], F32, tag="mu_sb")
        nc.vector.tensor_copy(out=mu_sb[:], in_=mups[:])
        musq = rt.tile([RH, BL], F32, tag="musq")
        nc.vector.tensor_tensor(out=musq[:], in0=mu_sb[:], in1=mu_sb[:],
                                op=OP.mult)
        var_bc = rt.tile([RH, BL], F32, tag="var_bc")
        nc.vector.tensor_tensor(out=var_bc[:], in0=msps[:], in1=musq[:],
                                op=OP.subtract)
        # rstd = exp(-0.5*ln(var+eps)) via the ACT tables (no DVE divide)
        lnv = rt.tile([RH, BL], F32, tag="lnv")
        nc.scalar.activation(out=lnv[:], in_=var_bc[:], func=AF.Ln,
                             bias=eps64[:])
        rstd_bc = rt.tile([RH, BL], F32, tag="rstd_bc")
        nc.scalar.activation(out=rstd_bc[:], in_=lnv[:], func=AF.Exp,
                             scale=-0.5)
        hn = rt.tile([RH, BL], F32R, tag="hn")
        nc.vector.tensor_tensor(out=hn[:], in0=hT_raw[:], in1=mu_sb[:],
                                op=OP.subtract)
        nc.vector.tensor_tensor(out=hn[:], in0=hn[:], in1=rstd_bc[:],
                                op=OP.mult)
        nc.vector.tensor_scalar(out=hn[:], in0=hn[:], scalar1=gln_c,
                                scalar2=bln_c, op0=OP.mult, op1=OP.add)
        nc.vector.tensor_single_scalar(out=hn[:], in_=hn[:], scalar=0.0,
                                       op=OP.max)

        k0_c((0, 1), 5)
        k0_c((0, 1), 6)

        ps3 = pst.tile([RH2, BL], F32, tag="ps")
        nc.tensor.matmul(out=ps3[:], lhsT=Q[:, 0:RH2], rhs=hn[:],
                         start=True, stop=True)
        h2T = rt.tile([RH2, BL], F32R, tag="h2T")
        nc.vector.tensor_scalar(out=h2T[:], in0=ps3[:], scalar1=b2_c,
                                scalar2=0.0, op0=OP.add, op1=OP.max)

        k0_c((0, 1), 7)
        drain_k0(0)
        drain_k0(1)

        ps4 = pst.tile([K, BL], F32, tag="ps")
        nc.tensor.matmul(out=ps4[:], lhsT=Q[0:RH2, RH2:RH2 + K], rhs=h2T[:],
                         start=True, stop=True)
        lgT = rt.tile([K, BL], F32, tag="lgT")
        nc.vector.tensor_scalar_add(out=lgT[:], in0=ps4[:], scalar1=b3p_c)

        # --- k0 bt-pair (2,3), logits/coef transposes overlapped
        k0_alloc((2, 3))
        k0_cgroup((2, 3), 0)

        lg = singles.tile([128, NBT, K], F32)
        for bt in range(NBT):
            ps5 = pst.tile([128, K], F32, tag="ps", name=f"ps5_{bt}")
            nc.tensor.transpose(out=ps5[:], in_=lgT[:, bt * 128:(bt + 1) * 128],
                                identity=ident[0:K, 0:K])
            nc.vector.tensor_copy(out=lg[:, bt, :], in_=ps5[:])

        k0_cgroup((2, 3), 1)

        # mask pipeline (all DVE/ACT, overlaps the k0 stream)
        s_all = rt.tile([128, NBT, K], F32, tag="s_all")
        nc.vector.tensor_tensor(out=s_all[:], in0=lg[:], in1=gum_sb, op=OP.add)
        soft_all = rt.tile([128, NBT, K], F32, tag="soft_all")
        nc.scalar.activation(out=soft_all[:], in_=s_all[:], func=AF.Sigmoid)

        # top-2 of 4 via minimax network (on logits; sigmoid is monotonic)
        a, b = lg[:, :, 0:1], lg[:, :, 1:2]
        c_, d_ = lg[:, :, 2:3], lg[:, :, 3:4]
        mab = rt.tile([128, NBT, 1], F32, tag="mab")
        nc.vector.tensor_tensor(out=mab[:], in0=a, in1=b, op=OP.max)
        mcd = rt.tile([128, NBT, 1], F32, tag="mcd")
        nc.vector.tensor_tensor(out=mcd[:], in0=c_, in1=d_, op=OP.max)
        nab = rt.tile([128, NBT, 1], F32, tag="nab")
        nc.vector.tensor_tensor(out=nab[:], in0=a, in1=b, op=OP.min)
        ncd = rt.tile([128, NBT, 1], F32, tag="ncd")
        nc.vector.tensor_tensor(out=ncd[:], in0=c_, in1=d_, op=OP.min)
        mmm = rt.tile([128, NBT, 1], F32, tag="mmm")
        nc.vector.tensor_tensor(out=mmm[:], in0=mab[:], in1=mcd[:], op=OP.min)
        m2a = rt.tile([128, NBT, 1], F32, tag="m2a")
        nc.vector.tensor_tensor(out=m2a[:], in0=nab[:], in1=ncd[:], op=OP.max)
        m2b = rt.tile([128, NBT, 1], F32, tag="m2b")
        nc.vector.tensor_tensor(out=m2b[:], in0=m2a[:], in1=mmm[:], op=OP.max)

        mnm = rt.tile([128, NBT, K], F32, tag="mnm")
        for kk in range(K):
            nc.vector.tensor_tensor(out=mnm[:, :, kk:kk + 1],
                                    in0=lg[:, :, kk:kk + 1],
                                    in1=m2b[:], op=OP.is_ge)
        msk = rt.tile([128, NBT, K], F32, tag="msk")
        nc.vector.tensor_tensor(out=msk[:], in0=soft_all[:], in1=mnm[:], op=OP.max)
        hm = rt.tile([128, NBT, K], F32, tag="hm")
        nc.vector.scalar_tensor_tensor(out=hm[:], in0=msk[:], scalar=0.5,
                                       in1=msk[:], op0=OP.is_gt, op1=OP.mult)
        for kk in range(K):
            nc.vector.tensor_scalar_mul(out=coef[:, :, kk:kk + 1],
                                        in0=hm[:, :, kk:kk + 1],
                                        scalar1=w4[:, kk:kk + 1])

        k0_cgroup((2, 3), 2)

        for bt in range(NBT):
            ps6 = pst.tile([K, 128], F32, tag="ps", name=f"ps6_{bt}")
            nc.tensor.transpose(out=ps6[:], in_=coef[:, bt, :], identity=ident[:])
            nc.vector.tensor_copy(out=coefT[:, bt, :], in_=ps6[:])

        k0_cgroup((2, 3), 3)
        drain_k0(2)
        drain_k0(3)

        # ---- k>=1 blocks (bt-major) + bias/coef0 fold interleaved ----
        def emit_bt(k, bt):
            pms = [psg.tile([128, HB], F32, tag="pm", name=f"pm{k}_{bt}_{hb}")
                   for hb in range(NHB)]

            def mms(hbs):
                for c in range(DCH):
                    for hb in hbs:
                        nc.tensor.matmul(
                            out=pms[hb][:],
                            lhsT=xs[k][:, c, bt * 128:(bt + 1) * 128],
                            rhs=ws[k][:, c, hb * HB:(hb + 1) * HB],
                            start=(c == 0), stop=(c == DCH - 1))

            def drain(hb):
                hsl = slice(hb * HB, (hb + 1) * HB)
                nc.vector.scalar_tensor_tensor(out=acc[:, bt, hsl],
                                               in0=pms[hb][:],
                                               scalar=coef[:, bt, k:k + 1],
                                               in1=acc[:, bt, hsl],
                                               op0=OP.mult, op1=OP.add)
                if k == K - 1:
                    # per-hb writes let the h0 half ship while h1 drains
                    nc.scalar.dma_start(
                        out=out_e[bt * 128:(bt + 1) * 128, hsl],
                        in_=acc[:, bt, hsl])

            if k == K - 1:
                # hb-major: h0 drains + ships while h1's matmuls still run,
                # shortening the post-last-matmul tail
                mms([0])
                drain(0)
                mms([1])
                if bt == NBT - 1:
                    # very last block: drain+ship hb1 in two 256-wide halves
                    # on two different queues, so the final stt/issue/payload
                    # /notify chain pipelines instead of serializing on one
                    # full-bank drain + one 256KB transfer
                    for half, eng in ((0, nc.scalar), (1, nc.sync)):
                        qsl = slice(HB + half * 256, HB + (half + 1) * 256)
                        psl = slice(half * 256, (half + 1) * 256)
                        nc.vector.scalar_tensor_tensor(
                            out=acc[:, bt, qsl], in0=pms[1][:, psl],
                            scalar=coef[:, bt, k:k + 1],
                            in1=acc[:, bt, qsl],
                            op0=OP.mult, op1=OP.add)
                        eng.dma_start(
                            out=out_e[bt * 128:(bt + 1) * 128, qsl],
                            in_=acc[:, bt, qsl])
                else:
                    drain(1)
            else:
                mms([0, 1])
                drain(0)
                drain(1)

        def emit_bias(bt):
            # acc = acc*coef0 + coefT^T @ b_enc
            for hb in range(NHB):
                hsl = slice(hb * HB, (hb + 1) * HB)
                pmb = pst.tile([128, HB], F32, tag="ps", name=f"pmb{bt}_{hb}")
                nc.tensor.matmul(out=pmb[:], lhsT=coefT[:, bt, :],
                                 rhs=benc_sb[:, hsl], start=True, stop=True)
                nc.vector.scalar_tensor_tensor(out=acc[:, bt, hsl],
                                               in0=acc[:, bt, hsl],
                                               scalar=coef[:, bt, 0:1],
                                               in1=pmb[:],
                                               op0=OP.mult, op1=OP.add)

        for bt in range(NBT):
            emit_bias(bt)
            emit_bt(1, bt)
        for k in range(2, K):
            for bt in range(NBT):
                emit_bt(k, bt)

    nc.compile()
    return nc


_NC = None


def _get_nc():
    global _NC
    if _NC is None:
        _NC = _build()
    return _NC


def _softmax(v):
    e = np.exp(v - np.max(v))
    return e / e.sum()


def _make_in_maps(inputs):
    f = {k: np.asarray(v) for k, v in inputs.items()}
    x_bf = f["x"].astype(_BF)                       # [K, B, D]
    W1P = (f["W1"].astype(np.float32).reshape(CCH, 128, RH)
           .transpose(1, 0, 2))                     # [128, CCH, RH]
    WP = np.ascontiguousarray(
        f["W_enc"].astype(_BF).reshape(K, DCH, 128, H).transpose(0, 2, 1, 3))
    w4 = _softmax(f["fusion_w"].astype(np.float64).ravel()).astype(np.float32)
    b3p = (f["b3"].astype(np.float32) + f["prior"].astype(np.float32)).ravel()
    Q = np.zeros((RH, RH2 + K), dtype=np.float32)
    Q[:, 0:RH2] = f["W2"].astype(np.float32)
    Q[0:RH2, RH2:RH2 + K] = f["W3"].astype(np.float32)
    PGbase = np.zeros((128, PGC), dtype=np.float32)
    PGbase[0:RH, 0] = f["b1"].astype(np.float32).ravel()
    PGbase[0:RH, 1] = f["g_ln"].astype(np.float32).ravel()
    PGbase[0:RH, 2] = f["beta_ln"].astype(np.float32).ravel()
    PGbase[0:RH2, 3] = f["b2"].astype(np.float32).ravel()
    PGbase[0:K, 4] = b3p
    PGbase[:, 5 + NBT * K:PGC] = w4[None, :]
    shared = {
        "Q": Q,
        "WP": WP,
        "b_encP": np.ascontiguousarray(f["b_enc"].astype(_BF)),
        "ident": np.eye(128, dtype=np.float32),
    }
    in_maps = []
    for i in range(N_CORES):
        sl = slice(i * BL, (i + 1) * BL)
        m = dict(shared)
        # ctxP[p, c, b] = context[b, c*128+p]; packed with W1P -> CW
        ctxP = (f["context"][sl].astype(np.float32).T.reshape(CCH, 128, BL)
                .transpose(1, 0, 2))
        m["CW"] = np.ascontiguousarray(
            np.concatenate([ctxP, W1P], axis=2))
        # xP[k, p, c, b] = x[k, b, c*128+p]
        m["xP"] = np.ascontiguousarray(
            x_bf[:, sl, :].transpose(0, 2, 1).reshape(K, DCH, 128, BL)
            .transpose(0, 2, 1, 3))
        # PG: base params + gumP[p, t, k] = gumbel[t*128+p, k]
        PG = PGbase.copy()
        PG[:, 5:5 + NBT * K] = (
            f["gumbel"][sl].astype(np.float32).reshape(NBT, 128, K)
            .transpose(1, 0, 2).reshape(128, NBT * K))
        m["PG"] = PG
        in_maps.append(m)
    return in_maps


def kernel(**inputs):
    nc = _get_nc()
    in_maps = _make_in_maps(inputs)
    res = run_bass_kernel_spmd(nc, in_maps, core_ids=list(range(N_CORES)))
    return np.concatenate([res.results[i]["out"] for i in range(N_CORES)],
                          axis=0)
